# revision 38
# baseline (speedup 1.0000x reference)
"""CTC loss kernel for Trainium2 (Bass/Tile), 8-core data parallel.

Linear-domain CTC forward DP.  Instead of the log-space recurrence
(max3 + softplus per step), alpha is kept as raw probabilities with
per-row periodic rescaling:

    alpha_t[s] = (alpha[s] + alpha[s-1] + m[s]*alpha[s-2]) * p_t[s]
    p_t[s]     = K * (y_pred[b,t,ext[b,s]] + EPS)       (K = e^LOGK)

Layout: 4 chunks x 32 batches across 128 partitions; each row holds
2 zero pads + 16 overlap states + 34 real states (CSZ even so state
parity == column parity on every row).  Per step, six narrow DVE ops:

    u_odd  = a_o + a_e            [128,25]  (chain head)
    vodd   = u_odd + sm_prev      [128,25]  (skip term; m[s]=0 at even s)
    u_even = a_e + a_o<<1         [128,25]  (off-chain)
    a'_ev  = u_even * p_even      [128,25]
    a'_od  = vodd  * p_odd        [128,25]  (chain tail)
    sm_nxt = vodd[s-2] * pm2      [128,25]  (lookahead skip term:
             pm2[s] = K*(y[ext[s-2]]+EPS)*m[s] = alpha'[s-2]*m[s]/v')

The dependence chain u_odd -> vodd -> a'_od is 3 same-engine hops
(~196 ns each: 86 processing + write-ack + semaphore); the other ops
ride the gaps, so a step runs at the latency floor ~590 ns.  Every 8
steps the overlap columns (alpha and sm) are re-synced from the
upstream chunk via PE partition-shift matmuls with a per-row scale
fixup F = exp(L[src]-L[dst]) computed 3 steps ahead; staggered by 4,
each row rescales its max to 2^30, folded free into the multiplies
via scalar_tensor_tensor; logacc accumulates the exact log of every
applied scale.  f32 flush of states ~115+ nats under a row's local
max loses negligible mass (HW-validated 3.8e-3 rel).

The per-symbol probabilities are gathered on-device as float32r
TensorEngine matmuls (1 cycle/row at moving dim 304; exact for
one-hot weights) against G matrices whose columns are grouped by
chunk (76 per chunk: 50 p + 25 pm2 + pad, overlap duplicated).  The
host passes y_pred pre-transposed with EPS added, so no on-device
transpose is needed and masked pm2 columns stay exactly zero.
Gather outputs stage through a DRAM scratch (written through the
Activation HWDGE queue, two batches per DMA) and stream back per-oct
as 4 contiguous DMAs on the SP queue into packed [row, t, 76] tiles;
pregather instances are emitted interleaved between DP steps so
in-order queues never head-of-line block the recurrence.
"""

import numpy as np

import concourse.bass as bass
import concourse.tile as tile
from concourse import bacc
from concourse import mybir
from concourse.ap import AP
from concourse.bass_utils import run_bass_kernel_spmd
from contextlib import ExitStack

B, T, C, L = 256, 1024, 128, 64
NCORES = 8
BPC = B // NCORES          # 32 batch rows per core
S = 2 * L + 1              # 129 extended states
NCH, CSZ = 4, 34           # state chunks per batch (CSZ even: parity-uniform)
W = 16                     # overlap states per chunk
NST = W + CSZ              # 50 computed states per row
NOD = NST // 2             # 25 odd / even state columns per row
RFR = 8                    # overlap refresh period (2 states/step drift)
BLANK = C - 1              # 127
EPS = 1e-7
LOGK = 4.85                # per-step compensation: p scaled by K=e^LOGK
TGTL2 = 30                 # rescale target 2^30 (centers f32 range)
OCT = 128                  # time steps per pregather matmul / stream tile
GRP = 76                   # gather cols per chunk: 50 p + 25 pm2 + 1 pad
SX2 = NCH * GRP            # 304 gather columns per batch
STW = 75                   # used stream cols per row per step

f32 = mybir.dt.float32
Alu = mybir.AluOpType
Act = mybir.ActivationFunctionType

# This kernel uses Copy / Ln / Exp activations, all present in the single
# "natural_log_exp_and_others" table.  Blank every other table (ids and
# positions preserved) so the placement pass settles on it once instead of
# thrashing 1.3us table loads around every Ln<->Exp transition.
_orig_get_act_tables = bacc.get_activation_tables


def _patched_get_act_tables(arch):
    tabs = _orig_get_act_tables(arch)
    keep = "natural_log_exp_and_others"
    if keep in tabs:
        tabs = {n: (fs if n == keep else set()) for n, fs in tabs.items()}
    return tabs


bacc.get_activation_tables = _patched_get_act_tables


def _sv(tile_, col, n, stride=1):
    """Strided single-free-dim view of a [128, X] tile."""
    base = tile_[:, col : col + 1]
    return AP(base.tensor, base.offset, [base.ap[0], [stride, n]])


def _build() -> bass.Bass:
    nc = bacc.Bacc()
    f32r = mybir.dt.float32r
    y_pred_t = nc.dram_tensor("y_pred_t", [BPC, C, T], f32r, kind="ExternalInput")
    g_in = nc.dram_tensor("g_all", [C, BPC * SX2], f32r, kind="ExternalInput")
    sh_in = nc.dram_tensor("shift32", [128, 128], f32, kind="ExternalInput")
    loss = nc.dram_tensor("loss", [BPC, 1], f32, kind="ExternalOutput")

    K = float(np.exp(np.float32(LOGK)))

    with tile.TileContext(nc) as tc, ExitStack() as ctx:
        persist = ctx.enter_context(tc.tile_pool(name="persist", bufs=1))
        tmp = ctx.enter_context(tc.tile_pool(name="tmp", bufs=2))
        ysb = ctx.enter_context(tc.tile_pool(name="ysb", bufs=5))
        ytp = ctx.enter_context(tc.tile_pool(name="ytp", bufs=5))
        pstream = ctx.enter_context(tc.tile_pool(name="pstream", bufs=3))
        psum_pp = ctx.enter_context(tc.tile_pool(name="psum_pp", bufs=6, space="PSUM"))
        psum_r = ctx.enter_context(tc.tile_pool(name="psum_r", bufs=1, space="PSUM"))
        dram = ctx.enter_context(tc.tile_pool(name="dram", bufs=1, space="DRAM"))

        # ---------- load static inputs ----------
        shift32 = persist.tile([128, 128], f32, tag="shift32")
        nc.sync.dma_start(out=shift32, in_=sh_in[:, :])
        g_all = persist.tile([C, BPC * SX2], f32r, tag="gall")

        def emit_g_slice(q):
            nc.sync.dma_start(
                out=g_all[:, q * 4 * SX2 : (q + 1) * 4 * SX2],
                in_=g_in[:, q * 4 * SX2 : (q + 1) * 4 * SX2])

        # ---------- pregather: P[b,t,:] = K*(y_pred[b,t,ext-ish] + EPS) ----
        # DRAM scratch per oct-group: [b][oct-in-group][t][c]
        OGROUPS = [[0], [1], [2, 3], [4, 5], [6, 7]]
        OCT_LOC = {}
        for gi, grp_ in enumerate(OGROUPS):
            for ii, o in enumerate(grp_):
                OCT_LOC[o] = (gi, ii)
        p_octg = [
            dram.tile([BPC, len(grp_), OCT, SX2], f32, tag=f"oct{gi}",
                      name=f"p_oct{gi}")
            for gi, grp_ in enumerate(OGROUPS)
        ]
        def emit_pregather_group(gi):
            for bp in range(BPC // 2):
                if gi == 0 and bp % 2 == 0:
                    emit_g_slice(bp // 2)
                emit_pregather_instance(gi, bp)

        def emit_pregather_instance(gi, bp):
            # one instance = 2 batches (2*bp, 2*bp+1) x the group's octs:
            # y_pred_t carries (y+EPS) transposed host-side, so the one-hot
            # gather matmul needs no on-device transpose and masked pm2
            # columns stay exactly zero.  Batch-pairing halves the HWDGE
            # acquires and queue issue slots per unit of work.
            octs = OGROUPS[gi]
            ng = len(octs)
            b0 = 2 * bp
            y2 = ysb.tile([C, 2, ng, OCT], f32r, tag="y")
            ysrc = y_pred_t[b0, :, octs[0] * OCT :]
            ysrc = AP(ysrc.tensor, ysrc.offset,
                      [[T, C], [C * T, 2], [OCT, ng], [1, OCT]])
            nc.sync.dma_start(out=y2, in_=ysrc)
            for i in range(ng):
                p_sb2 = ytp.tile([OCT, 2, SX2], f32, tag="psb")
                for j in range(2):
                    p_ps = psum_pp.tile([OCT, SX2], f32, tag="pp")
                    # float32r: exact for one-hot weights, 1 cycle/row at
                    # moving dim >= 256 (vs 4 for plain fp32)
                    nc.tensor.matmul(
                        p_ps, lhsT=y2[:, j, i, :],
                        rhs=g_all[:, (b0 + j) * SX2 : (b0 + j + 1) * SX2],
                        start=True, stop=True,
                    )
                    nc.scalar.activation(
                        out=p_sb2[:, j, :], in_=p_ps, func=Act.Copy, scale=K)
                # one DMA: src [t, b2, c] -> p_octg[gi][b0+j, i, t, c]
                d0 = p_octg[gi][b0, i, :, :]
                dst = AP(d0.tensor, d0.offset,
                         [[SX2, OCT], [ng * OCT * SX2, 2], [1, SX2]])
                nc.scalar.dma_start(out=dst, in_=p_sb2)

        # ---------- DP over time (linear domain, packed 4x32 partitions) ----
        # row p = 32k+b: chunk k of batch b; states 34k-16 .. 34k+33.
        # alpha tiles: cols 0,1 zero pads; col 2+j = state 34k-16+j.
        a_t = [
            persist.tile([128, NST + 2], f32, tag=f"alpha{i}", name=f"alpha{i}")
            for i in range(2)
        ]
        u_t = persist.tile([128, NST + 2], f32, tag="u")
        # vodd: col 0 pad; col 1+j = v at state col 3+2j (odd states)
        vo_t = persist.tile([128, NOD + 1], f32, tag="vodd")
        sm_t = [
            persist.tile([128, NOD + 1], f32, tag=f"sm{i}", name=f"sm{i}")
            for i in range(2)
        ]
        logacc = persist.tile([128, 1], f32, tag="logacc")
        smax_h = persist.tile([128, 1], f32, tag="smaxh")
        sc_t = persist.tile([128, 1], f32, tag="sc")
        rinv_t = persist.tile([128, 1], f32, tag="rinv")
        lns_t = persist.tile([128, 1], f32, tag="lns")
        fex_t = persist.tile([128, 1], f32, tag="fex")

        nc.vector.memset(a_t[0], 0.0)
        nc.vector.memset(a_t[1], 0.0)
        nc.vector.memset(u_t, 0.0)
        nc.vector.memset(vo_t, 0.0)
        nc.vector.memset(sm_t[0], 0.0)
        nc.vector.memset(sm_t[1], 0.0)
        nc.vector.memset(logacc, 0.0)
        nc.vector.memset(sc_t, 1.0)
        nc.vector.memset(rinv_t, 1.0)
        nc.vector.memset(fex_t, 1.0)

        # t=0 init: v(0)=1 at states 0 (col 18, even) and 1 (col 19, odd,
        # vodd j=8), rows 0:32 only; then the normal mul ops emit alpha(0).
        nc.vector.memset(u_t[0:32, 18:19], 1.0)
        nc.vector.memset(vo_t[0:32, 9:10], 1.0)

        def p_even(pt, tl):
            base = pt[:, tl, 0:1]
            return AP(base.tensor, base.offset, [base.ap[0], [2, NOD]])

        def p_odd(pt, tl):
            base = pt[:, tl, 1:2]
            return AP(base.tensor, base.offset, [base.ap[0], [2, NOD]])

        def pm2_ap(pt, tl, n):
            base = pt[:, tl, NST : NST + 1]
            return AP(base.tensor, base.offset, [base.ap[0], [1, n]])

        def step_muls(t, pt, tl, rescale):
            """alpha'(t) even/odd multiplies + lookahead skip term."""
            dst = a_t[t % 2]
            dev = _sv(dst, 2, NOD, 2)
            dod = _sv(dst, 3, NOD, 2)
            uev = _sv(u_t, 2, NOD, 2)
            vod = vo_t[:, 1 : 1 + NOD]
            if rescale:
                nc.vector.scalar_tensor_tensor(
                    out=dev, in0=uev, scalar=rinv_t[:, :], in1=p_even(pt, tl),
                    op0=Alu.mult, op1=Alu.mult)
                nc.vector.scalar_tensor_tensor(
                    out=dod, in0=vod, scalar=rinv_t[:, :], in1=p_odd(pt, tl),
                    op0=Alu.mult, op1=Alu.mult)
                nc.vector.scalar_tensor_tensor(
                    out=sm_t[t % 2][:, 1 : 1 + NOD],
                    in0=vo_t[:, 0:NOD], scalar=rinv_t[:, :],
                    in1=pm2_ap(pt, tl, NOD),
                    op0=Alu.mult, op1=Alu.mult)
            else:
                nc.vector.tensor_mul(out=dev, in0=uev, in1=p_even(pt, tl))
                nc.vector.tensor_mul(out=dod, in0=vod, in1=p_odd(pt, tl))
                nc.vector.tensor_mul(
                    out=sm_t[t % 2][:, 1 : 1 + NOD],
                    in0=vo_t[:, 0:NOD], in1=pm2_ap(pt, tl, NOD))

        def emit_fex():
            # F = exp(logacc[row-32] - logacc[row]); logacc is frozen
            # between rescales so this can run steps ahead of the refresh
            psL = psum_r.tile([128, 1], f32, tag="psL")
            nc.tensor.matmul(psL, lhsT=shift32, rhs=logacc, start=True, stop=True)
            dL = tmp.tile([128, 1], f32, tag="dL")
            nc.vector.tensor_sub(out=dL, in0=psL, in1=logacc)
            nc.scalar.activation(out=fex_t, in_=dL, func=Act.Exp)

        def refresh(t, pt, tl):
            """Re-sync overlap cols from upstream chunk with scale fixup."""
            dst = a_t[t % 2]
            # alpha overlap: cols 2:18 <- shift32(alpha cols 36:52) * F
            psA = psum_r.tile([128, W + 8], f32, tag="psA")
            nc.tensor.matmul(
                psA[:, 0:W], lhsT=shift32, rhs=dst[:, 2 + NST - W : 2 + NST],
                start=True, stop=True)
            # sm overlap: sm[t%2] cols 1:9 (state cols 3..17) need
            # vodd[src rows] cols 17:25 (state cols 35..51) * F * pm2
            nc.tensor.matmul(
                psA[:, W : W + 8], lhsT=shift32, rhs=vo_t[:, 17:25],
                start=True, stop=True)
            # full-partition ops (DVE patterns off partition 0 max 32 rows);
            # psA rows 0:32 are zero, so chunk 0 overlap correctly stays 0
            nc.vector.tensor_scalar_mul(
                dst[:, 2 : 2 + W], psA[:, 0:W], fex_t[:, :])
            nc.vector.scalar_tensor_tensor(
                out=sm_t[t % 2][:, 1:9],
                in0=psA[:, W : W + 8], scalar=fex_t[:, :],
                in1=pt[:, tl, NST : NST + 8],
                op0=Alu.mult, op1=Alu.mult)

        def emit_dp_oct(o, inst_sched=None):
            pt = pstream.tile([128, OCT, GRP], f32, tag="ps", name=f"pt{o%3}")
            gi, ii = OCT_LOC[o]
            thalves = ((0, OCT // 2), (OCT // 2, OCT)) if o == 0 else ((0, OCT),)
            for t0, t1 in thalves:
                for k in range(NCH):
                    nc.sync.dma_start(
                        out=pt[32 * k : 32 * (k + 1), t0:t1, :],
                        in_=p_octg[gi][:, ii, t0:t1, GRP * k : GRP * (k + 1)],
                    )
            for tl in range(OCT):
                t = o * OCT + tl
                if inst_sched is not None and tl in inst_sched:
                    gi, b = inst_sched[tl]
                    emit_pregather_instance(gi, b)
                if t == 0:
                    step_muls(0, pt, 0, False)
                    continue
                src = a_t[(t + 1) % 2]
                # u_odd[j] = a[odd col 3+2j] + a[even col 2+2j]  (chain head)
                nc.vector.tensor_add(
                    out=_sv(u_t, 3, NOD, 2),
                    in0=_sv(src, 3, NOD, 2), in1=_sv(src, 2, NOD, 2))
                # vodd = u_odd + sm_prev
                nc.vector.tensor_add(
                    out=vo_t[:, 1 : 1 + NOD],
                    in0=_sv(u_t, 3, NOD, 2), in1=sm_t[(t + 1) % 2][:, 1 : 1 + NOD])
                # u_even[j] = a[even col 2+2j] + a[odd col 1+2j]  (off-chain)
                nc.vector.tensor_add(
                    out=_sv(u_t, 2, NOD, 2),
                    in0=_sv(src, 2, NOD, 2), in1=_sv(src, 1, NOD, 2))
                rs = (t % RFR == 4 and t >= 12)
                step_muls(t, pt, tl, rs)
                if t % RFR == 5:
                    if t >= 13:
                        nc.vector.tensor_scalar_add(logacc, lns_t, logacc[:, :])
                    emit_fex()
                if t % RFR == 7:
                    nc.scalar.activation(out=lns_t, in_=sc_t, func=Act.Ln)
                if t % RFR == 6:
                    # rescale prep for t+6 (uses alpha(t), off the chain)
                    nc.vector.tensor_reduce(
                        out=smax_h, in_=a_t[t % 2][:, 2 : 2 + NST],
                        axis=mybir.AxisListType.X, op=Alu.max)
                if t % RFR == 7:
                    nc.vector.tensor_scalar(
                        out=sc_t, in0=smax_h,
                        scalar1=float(2.0 ** -TGTL2), scalar2=1.0,
                        op0=Alu.mult, op1=Alu.max)
                    nc.vector.reciprocal(out=rinv_t, in_=sc_t)
                if t % RFR == 0:
                    refresh(t, pt, tl)

        # emission interleave: group 0 upfront (lead-in), the rest spliced
        # one instance every few DP steps, each finishing before its
        # consumer oct starts: g1 in oct0, g2 in oct1, g3 in octs2-3,
        # g4 in octs 4-5.
        emit_pregather_group(0)
        sched = {o: {} for o in range(8)}
        for bp in range(BPC // 2):
            sched[0][6 * bp + 1] = (1, bp)        # deadline: step 128
            sched[1][7 * bp + 2] = (2, bp)        # deadline: step 256
            sched[2][8 * bp + 4] = (3, bp)        # deadline: step 512
            sched[3][8 * bp + 4] = (4, bp)        # deadline: step 768
        for o in range(8):
            sched[o] = {k: v for k, v in sched[o].items() if v is not None}
            emit_dp_oct(o, sched[o])

        # ---------- epilogue: loss = T*LOGK - logacc - ln(A[127]+A[128]) ---
        # states 127,128 = chunk 3 (rows 96:128) cols 43,44.
        a_fin = a_t[(T - 1) % 2]
        ssum = persist.tile([128, 1], f32, tag="ssum")
        nc.vector.tensor_add(
            out=ssum[96:128, :], in0=a_fin[96:128, 43:44], in1=a_fin[96:128, 44:45])
        nc.vector.tensor_scalar_max(ssum[96:128, :], ssum[96:128, :], 1e-37)
        lnv = persist.tile([128, 1], f32, tag="lnv")
        nc.scalar.activation(out=lnv[96:128, :], in_=ssum[96:128, :], func=Act.Ln)
        q1 = persist.tile([128, 1], f32, tag="q1")
        nc.vector.tensor_scalar_add(q1[96:128, :], lnv[96:128, :], logacc[96:128, :])
        out_t = persist.tile([128, 1], f32, tag="outt")
        nc.vector.tensor_scalar(
            out=out_t[96:128, :], in0=q1[96:128, :],
            scalar1=-1.0, scalar2=float(T) * float(np.float32(LOGK)),
            op0=Alu.mult, op1=Alu.add)
        nc.sync.dma_start(out=loss[:, :], in_=out_t[96:128, :])

    nc.finalize()
    return nc


def _host_prep_core(y_true_c: np.ndarray):
    """Per-batch gather matrix, grouped by chunk: group k (76 cols) =
    [50 p cols for states 34k-16..34k+33 | 25 pm2 cols for odd state
    cols 3+2j (pm2[s] = onehot(ext[s-2])*m[s]) | 1 zero pad]."""
    ext = np.full((BPC, S), BLANK, np.int32)
    ext[:, 1::2] = y_true_c
    m2 = np.zeros((BPC, S), np.bool_)
    m2[:, 3::2] = y_true_c[:, 1:] != y_true_c[:, :-1]
    cg = np.arange(C, dtype=np.int32)
    g = np.zeros((BPC, C, SX2), np.float32)
    for k in range(NCH):
        for j in range(NST):
            s = 34 * k - W + j
            if 0 <= s < S:
                g[:, :, GRP * k + j] = ext[:, s][:, None] == cg[None, :]
        for j in range(NOD):
            s = 34 * k - W + 1 + 2 * j      # state at odd col 3+2j
            if 2 <= s < S:
                sel = m2[:, s]
                if sel.any():
                    g[sel, :, GRP * k + NST + j] = (
                        ext[sel, s - 2][:, None] == cg[None, :]
                    ).astype(np.float32)
    return np.ascontiguousarray(g.transpose(1, 0, 2).reshape(C, BPC * SX2))


_NC = None
LAST_RESULT = None


def kernel(y_true: np.ndarray, y_pred: np.ndarray) -> np.ndarray:
    global _NC, LAST_RESULT
    if _NC is None:
        _NC = _build()
    y_true = np.asarray(y_true, dtype=np.int32)
    y_pred = np.asarray(y_pred, dtype=np.float32)
    # transposed + EPS-shifted copy: the gather then needs no on-device
    # transpose and masked gather columns stay exactly zero
    y_pred_teps = np.ascontiguousarray(
        y_pred.transpose(0, 2, 1) + np.float32(EPS))
    shift32 = np.zeros((128, 128), np.float32)
    # matmul(out, lhsT=shift32, rhs=x): out[m,f] = sum_k shift32[k,m] x[k,f]
    # want out[r] = x[r-32]: shift32[k, k+32] = 1
    for k in range(96):
        shift32[k, k + 32] = 1.0
    in_maps = []
    for i in range(NCORES):
        sl = slice(i * BPC, (i + 1) * BPC)
        g = _host_prep_core(y_true[sl])
        in_maps.append(
            {
                "y_pred_t": y_pred_teps[sl],
                "g_all": g,
                "shift32": shift32,
            }
        )
    res = run_bass_kernel_spmd(_NC, in_maps, core_ids=list(range(NCORES)))
    LAST_RESULT = res
    return np.concatenate([r["loss"] for r in res.results], axis=0)
